# revision 24
# baseline (speedup 1.0000x reference)
"""Multi-head self-attention Trainium2 kernel (8-core SPMD).

Problem: x[4,2048,1024] -> MHSA(16 heads, d=64) -> [4,2048,1024], f32.

Sharding: core = batch*2 + head_group. Each of the 8 cores handles one
batch (of 4) and one group of 8 heads (of 16): tensor-parallel over heads
x data-parallel over batch. The final FC contraction is split over head
groups; the host sums the two partial products per batch.

Math folds (exact up to float rounding):
 - b_k drops entirely: softmax over k is invariant to a per-q shift.
 - b_v contributes P@1 * b_v = b_v per row (softmax rows sum to 1), so
   b_v @ w_fc + b_fc is a constant [1024] vector added on the host.
 - b_q is folded into the Q^T projection as an extra rank-1 (ones-row)
   matmul on the device.
 - The softmax denominator Z comes free from the PV matmul: V is stored
   with a ones column appended per head, so row 64 of the PV psum is Z.

Device layouts (per core): all matmul operands live in SBUF as bf16
(DRAM inputs stay f32; gpsimd cast-DMAs convert in flight, enabling fast
weight loads on every matmul). PSUM accumulation is always f32.
 - xT [1024, 2048] (host-transposed)
 - Q^T, K^T [512, 2048] as 4 tiles [128, 2048] (head pair per tile)
 - V_aug 16 tiles [128, 520]: per head 65 cols = 64 V cols + ones col
 - S^T per (head-pair, q-chunk 512): k-tiles into a 2-bank [128, 1024]
   PSUM tile (both heads), one exp() per k-tile on ACT -> P^T bf16,
   consumed by PV matmuls (software-pipelined one k-tile ahead).
 - O^T [512, 2048] as 4 tiles [128, 2048]; FC -> y^T [1024, 2048] f32.

Schedule: projection for head-pair 0 + V runs first; attention for
head-pair hp overlaps the deferred projection of hp+1 and the FC of
finished q-chunks (emission order = scheduler priority), keeping the
PE dense and the HAM clock gate warm throughout.
"""

import numpy as np
from contextlib import ExitStack

import concourse.bass as bass
import concourse.tile as tile
import concourse.mybir as mybir
from concourse import bacc
from concourse._compat import with_exitstack
from concourse.bass_utils import run_bass_kernel_spmd

F32 = mybir.dt.float32
F32R = mybir.dt.float32r
BF16 = mybir.dt.bfloat16

B, S, E = 4, 2048, 1024
H, D = 16, 64
G = 2                      # head groups (tensor parallel)
HG = H // G                # 8 heads per core
DG = HG * D                # 512 = head-group width
NCORES = B * G             # 8

DT_X = BF16                # xT / wq / wk / wv / bq / ones (proj inputs)
DT_PROJ = BF16             # wfc / OT (FC operands)
DT_ATTN = BF16             # QT / KT / V_aug / PT

EC = E // 128              # 8  e-chunks
SC = S // 512              # 4  s-chunks (q-chunks)
ST = S // 128              # 16 s-tiles (k-tiles)
DTL = DG // 128            # 4  d-tiles (head pairs)
NT = E // 128              # 8  n-tiles of output


def _np_dt(dt):
    return np.dtype(mybir.dt.np(dt))


@with_exitstack
def _emit(ctx: ExitStack, tc: tile.TileContext, io: dict):
    nc = tc.nc
    xT_d, wq_d, wk_d, wv_d, bq_d, wfc_d, yT_d = (
        io["xT"], io["wq"], io["wk"], io["wv"], io["bq"], io["wfc"], io["yT"])

    sbW = ctx.enter_context(tc.tile_pool(name="sbW", bufs=1))
    sbP = ctx.enter_context(tc.tile_pool(name="sbP", bufs=1))
    xt_pool = ctx.enter_context(tc.tile_pool(name="xt", bufs=24))
    pt_pool = ctx.enter_context(tc.tile_pool(name="pt", bufs=8))
    ev_pool = ctx.enter_context(tc.tile_pool(name="ev", bufs=6))
    nrm_pool = ctx.enter_context(tc.tile_pool(name="nrm", bufs=3))
    s_ps = ctx.enter_context(tc.tile_pool(name="sps", bufs=3, space="PSUM"))
    o_ps = ctx.enter_context(tc.tile_pool(name="ops", bufs=2, space="PSUM"))

    def load_xt(sc):
        s0 = sc * 512
        xt = []
        for ec in range(EC):
            t = xt_pool.tile([128, 512], DT_X, name=f"xt{ec}", tag="xt")
            nc.gpsimd.dma_start(t[:], xT_d[ec * 128:(ec + 1) * 128, s0:s0 + 512])
            xt.append(t)
        return xt

    # ---- xt chunk 0 + wq interleaved: the first projection group's
    # operands (xt0[ec], wq[ec]) land pairwise ASAP ----
    xt0 = []
    wq_t, wk_t, wv_t = [], [], []
    for ec in range(EC):
        t = xt_pool.tile([128, 512], DT_X, name=f"xt{ec}", tag="xt")
        nc.gpsimd.dma_start(t[:], xT_d[ec * 128:(ec + 1) * 128, 0:512])
        xt0.append(t)
        w = sbW.tile([128, DG], DT_X, name=f"wq{ec}", tag=f"wq{ec}")
        nc.gpsimd.dma_start(w[:], wq_d[ec * 128:(ec + 1) * 128, :])
        wq_t.append(w)
    for ec in range(EC):
        t = sbW.tile([128, DG], DT_X, name=f"wk{ec}", tag=f"wk{ec}")
        nc.gpsimd.dma_start(t[:], wk_d[ec * 128:(ec + 1) * 128, :])
        wk_t.append(t)
    # bq as [128, 4]: column dt holds the 128 bias values of head-pair dt
    # (f32 so it can be the tensor_scalar operand on the f32 psum)
    bq_t = sbW.tile([128, DTL], F32, name="bq", tag="bq")
    nc.sync.dma_start(bq_t[:], bq_d.rearrange("o (a p) -> (o p) a", p=128))
    for ec in range(EC):
        t = sbW.tile([128, DG], DT_X, name=f"wv{ec}", tag=f"wv{ec}")
        nc.gpsimd.dma_start(t[:], wv_d[ec * 128:(ec + 1) * 128, :])
        wv_t.append(t)

    # ---- persistent activations ----
    QT = [sbP.tile([128, S], DT_ATTN, name=f"QT{i}", tag=f"QT{i}")
          for i in range(DTL)]
    KT = [sbP.tile([128, S], DT_ATTN, name=f"KT{i}", tag=f"KT{i}")
          for i in range(DTL)]
    VA = [sbP.tile([128, HG * 65], DT_ATTN, name=f"VA{i}", tag=f"VA{i}")
          for i in range(ST)]
    OT = [sbP.tile([128, S], DT_PROJ, name=f"OT{i}", tag=f"OT{i}")
          for i in range(DTL)]

    # ones columns of V_aug (col 64 of each head's 65-col block)
    for st in range(ST):
        va3 = VA[st].rearrange("p (h c) -> p h c", c=65)
        nc.vector.memset(va3[:, :, 64:65], 1.0)

    # ---- projection pieces ----
    def emit_qk(dt_i, sc, xt):
        """Q^T (with bias) and K^T for one head-pair tile, one s-chunk."""
        s0 = sc * 512
        dsl = slice(dt_i * 128, (dt_i + 1) * 128)
        pq = s_ps.tile([128, 512], F32, name="pq", tag="ps")
        for ec in range(EC):
            nc.tensor.matmul(pq[:], wq_t[ec][:, dsl], xt[ec][:],
                             start=(ec == 0), stop=(ec == EC - 1))
        nc.vector.tensor_scalar_add(QT[dt_i][:, s0:s0 + 512], pq[:],
                                    bq_t[:, dt_i:dt_i + 1])
        pk = s_ps.tile([128, 512], F32, name="pk", tag="ps")
        for ec in range(EC):
            nc.tensor.matmul(pk[:], wk_t[ec][:, dsl], xt[ec][:],
                             start=(ec == 0), stop=(ec == EC - 1))
        nc.vector.tensor_copy(KT[dt_i][:, s0:s0 + 512], pk[:])

    def emit_v(sc, xt):
        for st_l in range(4):
            st = sc * 4 + st_l
            ssl = slice(st_l * 128, (st_l + 1) * 128)
            pv = s_ps.tile([128, 512], F32, name="pv", tag="ps")
            for ec in range(EC):
                nc.tensor.matmul(pv[:], xt[ec][:, ssl], wv_t[ec][:],
                                 start=(ec == 0), stop=(ec == EC - 1))
            va3 = VA[st].rearrange("p (h c) -> p h c", c=65)
            pv3 = pv.rearrange("p (h d) -> p h d", d=64)
            nc.vector.tensor_copy(va3[:, :, 0:64], pv3[:])

    def emit_fc(sc):
        s0 = sc * 512
        for nt in range(NT):
            nsl = slice(nt * 128, (nt + 1) * 128)
            py = s_ps.tile([128, 512], F32, name="py", tag="ps")
            for dt_i in range(DTL):
                nc.tensor.matmul(py[:], wfc_t[dt_i][:, nsl],
                                 OT[dt_i][:, s0:s0 + 512],
                                 start=(dt_i == 0), stop=(dt_i == DTL - 1))
            yv = ev_pool.tile([128, 512], F32, name="yv", tag="yv")
            nc.vector.tensor_copy(yv[:], py[:])
            nc.sync.dma_start(yT_d[nt * 128:(nt + 1) * 128, s0:s0 + 512],
                              yv[:])

    def emit_attn(hp, qc):
        q0 = qc * 512
        po = [o_ps.tile([65, 512], F32, name=f"po{p}", tag="po")
              for p in range(2)]
        # software-pipelined: S/exp of k-tile kt are emitted one step
        # ahead of PV of k-tile kt-1, so the PE stream has independent
        # S work in hand whenever a PV would wait on the exp.
        pend = None  # (kt, pt_t) awaiting PV
        for kt in range(ST):
            k0 = kt * 128
            ps_t = s_ps.tile([128, 1024], F32, name="ps", tag="ps")
            for p in range(2):
                psl = slice(p * 64, (p + 1) * 64)
                nc.tensor.matmul(ps_t[:, p * 512:(p + 1) * 512],
                                 KT[hp][psl, k0:k0 + 128],
                                 QT[hp][psl, q0:q0 + 512],
                                 start=True, stop=True)
            pt_t = pt_pool.tile([128, 1024], DT_ATTN, name="ptt", tag="ptt")
            nc.scalar.activation(pt_t[:], ps_t[:],
                                 mybir.ActivationFunctionType.Exp)
            if pend is not None:
                pkt, ppt = pend
                for p in range(2):
                    h_l = hp * 2 + p
                    nc.tensor.matmul(po[p][:],
                                     VA[pkt][:, h_l * 65:(h_l + 1) * 65],
                                     ppt[:, p * 512:(p + 1) * 512],
                                     start=(pkt == 0), stop=False)
            pend = (kt, pt_t)
        pkt, ppt = pend
        for p in range(2):
            h_l = hp * 2 + p
            nc.tensor.matmul(po[p][:],
                             VA[pkt][:, h_l * 65:(h_l + 1) * 65],
                             ppt[:, p * 512:(p + 1) * 512],
                             start=False, stop=True)
        for p in range(2):
            # quick-evacuate PV psum to release the bank, then normalize
            posb = nrm_pool.tile([65, 512], F32, name="posb", tag="posb")
            nc.vector.tensor_copy(posb[:], po[p][:])
            # custom DVE ops and partition_broadcast read the tensor's
            # partition 0 regardless of AP offset -> move Z via DMA first
            zrow = nrm_pool.tile([1, 512], F32, name="zrow", tag="zrow")
            nc.sync.dma_start(zrow[:], posb[64:65, :])
            rz = nrm_pool.tile([1, 512], F32, name="rz", tag="rz")
            nc.vector.reciprocal_approx_fast(rz[:], zrow[:])
            rzb = nrm_pool.tile([64, 512], F32, name="rzb", tag="rzb")
            nc.gpsimd.partition_broadcast(rzb[:], rz[0:1, :])
            if p == 0:
                nc.vector.tensor_mul(OT[hp][0:64, q0:q0 + 512],
                                     posb[0:64, :], rzb[:])
            else:
                tmp = nrm_pool.tile([64, 512], DT_PROJ,
                                    name="otmp", tag="otmp")
                nc.vector.tensor_mul(tmp[:], posb[0:64, :], rzb[:])
                # DVE cannot shift partitions; DMA moves rows 0:64
                # into OT rows 64:128.
                nc.sync.dma_start(OT[hp][64:128, q0:q0 + 512], tmp[:])

    # ---- pass A: V (all heads) + Q/K for head-pair 0 ----
    for sc in range(SC):
        xt = xt0 if sc == 0 else load_xt(sc)
        emit_qk(0, sc, xt)
        emit_v(sc, xt)

    # wfc loads deferred past pass A: first FC use is in the hp3 window,
    # and these 2MB would otherwise crowd the DMA-paced startup.
    wfc_t = []
    for dt_i in range(DTL):
        t = sbW.tile([128, E], DT_PROJ, name=f"wfc{dt_i}", tag=f"wfc{dt_i}")
        nc.gpsimd.dma_start(t[:], wfc_d[dt_i * 128:(dt_i + 1) * 128, :])
        wfc_t.append(t)

    # ---- attention interleaved with deferred projections ----
    # Attention for head-pair hp runs while the projection for head-pair
    # hp+1 (emitted just after, lower priority) fills PE gaps.
    for hp in range(DTL):
        for qc in range(SC):
            emit_attn(hp, qc)
            if hp == DTL - 1 and qc >= 1:
                # FC for the previous q-chunk: emitted after this chunk's
                # attention so attention keeps scheduling priority and FC
                # fills PE gaps.
                emit_fc(qc - 1)
        if hp + 1 < DTL:
            for sc in range(SC):
                xt = load_xt(sc)
                emit_qk(hp + 1, sc, xt)
    emit_fc(SC - 1)

_CACHE = {}


def _build():
    if "nc" in _CACHE:
        return _CACHE["nc"]
    nc = bacc.Bacc("TRN2", target_bir_lowering=False, debug=False)
    io = {
        "xT": nc.dram_tensor("xT", [E, S], F32, kind="ExternalInput").ap(),
        "wq": nc.dram_tensor("wq", [E, DG], F32, kind="ExternalInput").ap(),
        "wk": nc.dram_tensor("wk", [E, DG], F32, kind="ExternalInput").ap(),
        "wv": nc.dram_tensor("wv", [E, DG], F32, kind="ExternalInput").ap(),
        "bq": nc.dram_tensor("bq", [1, DG], F32, kind="ExternalInput").ap(),
        "wfc": nc.dram_tensor("wfc", [DG, E], F32,
                              kind="ExternalInput").ap(),
        "yT": nc.dram_tensor("yT", [E, S], F32, kind="ExternalOutput").ap(),
    }
    with tile.TileContext(nc) as tc:
        _emit(tc, io)
    nc.compile()
    _CACHE["nc"] = nc
    return nc


def make_in_maps(x, w_qkv, b_qkv, w_fc):
    """Host-side sharding: returns per-core input dicts (core = b*G + g)."""
    x = np.asarray(x, dtype=np.float32)
    w_qkv = np.asarray(w_qkv, dtype=np.float32)
    b_qkv = np.asarray(b_qkv, dtype=np.float32)
    w_fc = np.asarray(w_fc, dtype=np.float32)
    npdt = np.float32
    in_maps = []
    for b in range(B):
        xTb = np.ascontiguousarray(x[b].T).astype(npdt)
        for g in range(G):
            gs = slice(g * DG, (g + 1) * DG)
            in_maps.append({
                "xT": xTb,
                "wq": np.ascontiguousarray(
                    w_qkv[:, 0 * E:1 * E][:, gs] * (1.0 / np.sqrt(D))
                ).astype(npdt),
                "wk": np.ascontiguousarray(w_qkv[:, 1 * E:2 * E][:, gs]).astype(npdt),
                "wv": np.ascontiguousarray(w_qkv[:, 2 * E:3 * E][:, gs]).astype(npdt),
                "bq": np.ascontiguousarray(
                    b_qkv[0 * E:1 * E][gs][None, :] * (1.0 / np.sqrt(D))
                ).astype(npdt),
                "wfc": np.ascontiguousarray(w_fc[gs, :]).astype(npdt),
            })
    return in_maps


def gather(results, b_qkv, w_fc, b_fc):
    """Host-side unshard: sum group partials, transpose, add const bias."""
    b_qkv = np.asarray(b_qkv, dtype=np.float32)
    w_fc = np.asarray(w_fc, dtype=np.float32)
    b_fc = np.asarray(b_fc, dtype=np.float32)
    cbias = (b_qkv[2 * E:3 * E].astype(np.float64) @ w_fc.astype(np.float64)
             + b_fc.astype(np.float64)).astype(np.float32)
    y = np.empty((B, S, E), np.float32)
    for b in range(B):
        yT = results[b * G]["yT"] + results[b * G + 1]["yT"]
        y[b] = yT.T + cbias[None, :]
    return y


def kernel(x, w_qkv, b_qkv, w_fc, b_fc, _trace=False, _tmpdir=None):
    nc = _build()
    in_maps = make_in_maps(x, w_qkv, b_qkv, w_fc)
    res = run_bass_kernel_spmd(nc, in_maps, list(range(NCORES)),
                               trace=_trace, tmpdir=_tmpdir)
    y = gather(res.results, b_qkv, w_fc, b_fc)
    kernel.last_exec_time_ns = res.exec_time_ns
    kernel.last_res = res
    return y


# revision 25
# speedup vs baseline: 1.0653x; 1.0653x over previous
"""Multi-head self-attention Trainium2 kernel (8-core SPMD).

Problem: x[4,2048,1024] -> MHSA(16 heads, d=64) -> [4,2048,1024], f32.

Sharding: core = batch*2 + head_group. Each of the 8 cores handles one
batch (of 4) and one group of 8 heads (of 16): tensor-parallel over heads
x data-parallel over batch. The final FC contraction is split over head
groups; the host sums the two partial products per batch.

Math folds (exact up to float rounding):
 - b_k drops entirely: softmax over k is invariant to a per-q shift.
 - b_v contributes P@1 * b_v = b_v per row (softmax rows sum to 1), so
   b_v @ w_fc + b_fc is a constant [1024] vector added on the host.
 - b_q is folded into the Q^T projection as an extra rank-1 (ones-row)
   matmul on the device.
 - The softmax denominator Z comes free from the PV matmul: V is stored
   with a ones column appended per head, so row 64 of the PV psum is Z.

Device layouts (per core): all matmul operands live in SBUF as bf16
(DRAM inputs stay f32; gpsimd cast-DMAs convert in flight, enabling fast
weight loads on every matmul). PSUM accumulation is always f32.
 - xT [1024, 2048] (host-transposed)
 - Q^T, K^T [512, 2048] as 4 tiles [128, 2048] (head pair per tile)
 - V_aug 16 tiles [128, 520]: per head 65 cols = 64 V cols + ones col
 - S^T per (head-pair, q-chunk 512): k-tiles into a 2-bank [128, 1024]
   PSUM tile (both heads), one exp() per k-tile on ACT -> P^T bf16,
   consumed by PV matmuls (software-pipelined one k-tile ahead).
 - O^T [512, 2048] as 4 tiles [128, 2048]; FC -> y^T [1024, 2048] f32.

Schedule: projection for head-pair 0 + V runs first; attention for
head-pair hp overlaps the deferred projection of hp+1 and the FC of
finished q-chunks (emission order = scheduler priority), keeping the
PE dense and the HAM clock gate warm throughout.
"""

import numpy as np
from contextlib import ExitStack

import concourse.bass as bass
import concourse.tile as tile
import concourse.mybir as mybir
from concourse import bacc
from concourse._compat import with_exitstack
from concourse.bass_utils import run_bass_kernel_spmd

F32 = mybir.dt.float32
F32R = mybir.dt.float32r
BF16 = mybir.dt.bfloat16

B, S, E = 4, 2048, 1024
H, D = 16, 64
G = 2                      # head groups (tensor parallel)
HG = H // G                # 8 heads per core
DG = HG * D                # 512 = head-group width
NCORES = B * G             # 8

DT_X = BF16                # xT / wq / wk / wv / bq / ones (proj inputs)
DT_PROJ = BF16             # wfc / OT (FC operands)
DT_ATTN = BF16             # QT / KT / V_aug / PT

EC = E // 128              # 8  e-chunks
SC = S // 512              # 4  s-chunks (q-chunks)
ST = S // 128              # 16 s-tiles (k-tiles)
DTL = DG // 128            # 4  d-tiles (head pairs)
NT = E // 128              # 8  n-tiles of output


def _np_dt(dt):
    return np.dtype(mybir.dt.np(dt))


@with_exitstack
def _emit(ctx: ExitStack, tc: tile.TileContext, io: dict):
    nc = tc.nc
    xT_d, wq_d, wk_d, wv_d, bq_d, wfc_d, yT_d = (
        io["xT"], io["wq"], io["wk"], io["wv"], io["bq"], io["wfc"], io["yT"])

    sbW = ctx.enter_context(tc.tile_pool(name="sbW", bufs=1))
    sbP = ctx.enter_context(tc.tile_pool(name="sbP", bufs=1))
    xt_pool = ctx.enter_context(tc.tile_pool(name="xt", bufs=24))
    pt_pool = ctx.enter_context(tc.tile_pool(name="pt", bufs=8))
    ev_pool = ctx.enter_context(tc.tile_pool(name="ev", bufs=6))
    nrm_pool = ctx.enter_context(tc.tile_pool(name="nrm", bufs=3))
    mm_ps = ctx.enter_context(tc.tile_pool(name="mmps", bufs=2, space="PSUM"))
    s_ps = ctx.enter_context(tc.tile_pool(name="sps", bufs=2, space="PSUM"))
    o_ps = ctx.enter_context(tc.tile_pool(name="ops", bufs=2, space="PSUM"))

    def load_xt(sc):
        s0 = sc * 512
        xt = []
        for ec in range(EC):
            t = xt_pool.tile([128, 512], DT_X, name=f"xt{ec}", tag="xt")
            nc.gpsimd.dma_start(t[:], xT_d[ec * 128:(ec + 1) * 128, s0:s0 + 512])
            xt.append(t)
        return xt

    # ---- xt chunk 0 + wq interleaved: the first projection group's
    # operands (xt0[ec], wq[ec]) land pairwise ASAP ----
    xt0 = []
    wq_t, wk_t, wv_t = [], [], []
    for ec in range(EC):
        t = xt_pool.tile([128, 512], DT_X, name=f"xt{ec}", tag="xt")
        nc.gpsimd.dma_start(t[:], xT_d[ec * 128:(ec + 1) * 128, 0:512])
        xt0.append(t)
        w = sbW.tile([128, DG], DT_X, name=f"wq{ec}", tag=f"wq{ec}")
        nc.gpsimd.dma_start(w[:], wq_d[ec * 128:(ec + 1) * 128, :])
        wq_t.append(w)
    for ec in range(EC):
        t = sbW.tile([128, DG], DT_X, name=f"wk{ec}", tag=f"wk{ec}")
        nc.gpsimd.dma_start(t[:], wk_d[ec * 128:(ec + 1) * 128, :])
        wk_t.append(t)
    # bq as [128, 4]: column dt holds the 128 bias values of head-pair dt
    # (f32 so it can be the tensor_scalar operand on the f32 psum)
    bq_t = sbW.tile([128, DTL], F32, name="bq", tag="bq")
    nc.sync.dma_start(bq_t[:], bq_d.rearrange("o (a p) -> (o p) a", p=128))
    for ec in range(EC):
        t = sbW.tile([128, DG], DT_X, name=f"wv{ec}", tag=f"wv{ec}")
        nc.gpsimd.dma_start(t[:], wv_d[ec * 128:(ec + 1) * 128, :])
        wv_t.append(t)

    # ---- persistent activations ----
    QT = [sbP.tile([128, S], DT_ATTN, name=f"QT{i}", tag=f"QT{i}")
          for i in range(DTL)]
    KT = [sbP.tile([128, S], DT_ATTN, name=f"KT{i}", tag=f"KT{i}")
          for i in range(DTL)]
    VA = [sbP.tile([128, HG * 65], DT_ATTN, name=f"VA{i}", tag=f"VA{i}")
          for i in range(ST)]
    OT = [sbP.tile([128, S], DT_PROJ, name=f"OT{i}", tag=f"OT{i}")
          for i in range(DTL)]

    # ones columns of V_aug (col 64 of each head's 65-col block)
    for st in range(ST):
        va3 = VA[st].rearrange("p (h c) -> p h c", c=65)
        nc.vector.memset(va3[:, :, 64:65], 1.0)

    # ---- projection pieces ----
    def emit_qk(dt_i, sc, xt):
        """Q^T (with bias) and K^T for one head-pair tile, one s-chunk."""
        s0 = sc * 512
        dsl = slice(dt_i * 128, (dt_i + 1) * 128)
        pq = mm_ps.tile([128, 512], F32, name="pq", tag="mm")
        for ec in range(EC):
            nc.tensor.matmul(pq[:], wq_t[ec][:, dsl], xt[ec][:],
                             start=(ec == 0), stop=(ec == EC - 1))
        nc.vector.tensor_scalar_add(QT[dt_i][:, s0:s0 + 512], pq[:],
                                    bq_t[:, dt_i:dt_i + 1])
        pk = mm_ps.tile([128, 512], F32, name="pk", tag="mm")
        for ec in range(EC):
            nc.tensor.matmul(pk[:], wk_t[ec][:, dsl], xt[ec][:],
                             start=(ec == 0), stop=(ec == EC - 1))
        nc.vector.tensor_copy(KT[dt_i][:, s0:s0 + 512], pk[:])

    def emit_v(sc, xt):
        for st_l in range(4):
            st = sc * 4 + st_l
            ssl = slice(st_l * 128, (st_l + 1) * 128)
            pv = mm_ps.tile([128, 512], F32, name="pv", tag="mm")
            for ec in range(EC):
                nc.tensor.matmul(pv[:], xt[ec][:, ssl], wv_t[ec][:],
                                 start=(ec == 0), stop=(ec == EC - 1))
            va3 = VA[st].rearrange("p (h c) -> p h c", c=65)
            pv3 = pv.rearrange("p (h d) -> p h d", d=64)
            nc.vector.tensor_copy(va3[:, :, 0:64], pv3[:])

    def emit_fc(sc):
        s0 = sc * 512
        for nt in range(NT):
            nsl = slice(nt * 128, (nt + 1) * 128)
            py = mm_ps.tile([128, 512], F32, name="py", tag="mm")
            for dt_i in range(DTL):
                nc.tensor.matmul(py[:], wfc_t[dt_i][:, nsl],
                                 OT[dt_i][:, s0:s0 + 512],
                                 start=(dt_i == 0), stop=(dt_i == DTL - 1))
            yv = ev_pool.tile([128, 512], F32, name="yv", tag="yv")
            nc.vector.tensor_copy(yv[:], py[:])
            nc.sync.dma_start(yT_d[nt * 128:(nt + 1) * 128, s0:s0 + 512],
                              yv[:])

    def emit_attn(hp, qc):
        q0 = qc * 512
        po = [o_ps.tile([65, 512], F32, name=f"po{p}", tag="po")
              for p in range(2)]
        # software-pipelined: S/exp of k-tile kt are emitted one step
        # ahead of PV of k-tile kt-1, so the PE stream has independent
        # S work in hand whenever a PV would wait on the exp.
        pend = None  # (kt, pt_t) awaiting PV
        for kt in range(ST):
            k0 = kt * 128
            ps_t = s_ps.tile([128, 1024], F32, name="ps", tag="ps")
            for p in range(2):
                psl = slice(p * 64, (p + 1) * 64)
                nc.tensor.matmul(ps_t[:, p * 512:(p + 1) * 512],
                                 KT[hp][psl, k0:k0 + 128],
                                 QT[hp][psl, q0:q0 + 512],
                                 start=True, stop=True)
            pt_t = pt_pool.tile([128, 1024], DT_ATTN, name="ptt", tag="ptt")
            nc.scalar.activation(pt_t[:], ps_t[:],
                                 mybir.ActivationFunctionType.Exp)
            if pend is not None:
                pkt, ppt = pend
                for p in range(2):
                    h_l = hp * 2 + p
                    nc.tensor.matmul(po[p][:],
                                     VA[pkt][:, h_l * 65:(h_l + 1) * 65],
                                     ppt[:, p * 512:(p + 1) * 512],
                                     start=(pkt == 0), stop=False)
            pend = (kt, pt_t)
        pkt, ppt = pend
        for p in range(2):
            h_l = hp * 2 + p
            nc.tensor.matmul(po[p][:],
                             VA[pkt][:, h_l * 65:(h_l + 1) * 65],
                             ppt[:, p * 512:(p + 1) * 512],
                             start=False, stop=True)
        for p in range(2):
            # quick-evacuate PV psum to release the bank, then normalize
            posb = nrm_pool.tile([65, 512], F32, name="posb", tag="posb")
            nc.vector.tensor_copy(posb[:], po[p][:])
            # custom DVE ops and partition_broadcast read the tensor's
            # partition 0 regardless of AP offset -> move Z via DMA first
            zrow = nrm_pool.tile([1, 512], F32, name="zrow", tag="zrow")
            nc.sync.dma_start(zrow[:], posb[64:65, :])
            rz = nrm_pool.tile([1, 512], F32, name="rz", tag="rz")
            nc.vector.reciprocal_approx_fast(rz[:], zrow[:])
            rzb = nrm_pool.tile([64, 512], F32, name="rzb", tag="rzb")
            nc.gpsimd.partition_broadcast(rzb[:], rz[0:1, :])
            if p == 0:
                nc.vector.tensor_mul(OT[hp][0:64, q0:q0 + 512],
                                     posb[0:64, :], rzb[:])
            else:
                tmp = nrm_pool.tile([64, 512], DT_PROJ,
                                    name="otmp", tag="otmp")
                nc.vector.tensor_mul(tmp[:], posb[0:64, :], rzb[:])
                # DVE cannot shift partitions; DMA moves rows 0:64
                # into OT rows 64:128.
                nc.sync.dma_start(OT[hp][64:128, q0:q0 + 512], tmp[:])

    # ---- pass A: V (all heads) + Q/K for head-pair 0 ----
    for sc in range(SC):
        xt = xt0 if sc == 0 else load_xt(sc)
        emit_qk(0, sc, xt)
        emit_v(sc, xt)

    # wfc loads deferred past pass A: first FC use is in the hp3 window,
    # and these 2MB would otherwise crowd the DMA-paced startup.
    wfc_t = []
    for dt_i in range(DTL):
        t = sbW.tile([128, E], DT_PROJ, name=f"wfc{dt_i}", tag=f"wfc{dt_i}")
        nc.gpsimd.dma_start(t[:], wfc_d[dt_i * 128:(dt_i + 1) * 128, :])
        wfc_t.append(t)

    # ---- attention interleaved with deferred projections ----
    # Attention for head-pair hp runs while the projection for head-pair
    # hp+1 (emitted just after, lower priority) fills PE gaps.
    for hp in range(DTL):
        for qc in range(SC):
            emit_attn(hp, qc)
            if hp == DTL - 1 and qc >= 1:
                # FC for the previous q-chunk: emitted after this chunk's
                # attention so attention keeps scheduling priority and FC
                # fills PE gaps.
                emit_fc(qc - 1)
        if hp + 1 < DTL:
            for sc in range(SC):
                xt = load_xt(sc)
                emit_qk(hp + 1, sc, xt)
    emit_fc(SC - 1)

_CACHE = {}


def _build():
    if "nc" in _CACHE:
        return _CACHE["nc"]
    nc = bacc.Bacc("TRN2", target_bir_lowering=False, debug=False)
    io = {
        "xT": nc.dram_tensor("xT", [E, S], F32, kind="ExternalInput").ap(),
        "wq": nc.dram_tensor("wq", [E, DG], F32, kind="ExternalInput").ap(),
        "wk": nc.dram_tensor("wk", [E, DG], F32, kind="ExternalInput").ap(),
        "wv": nc.dram_tensor("wv", [E, DG], F32, kind="ExternalInput").ap(),
        "bq": nc.dram_tensor("bq", [1, DG], F32, kind="ExternalInput").ap(),
        "wfc": nc.dram_tensor("wfc", [DG, E], F32,
                              kind="ExternalInput").ap(),
        "yT": nc.dram_tensor("yT", [E, S], F32, kind="ExternalOutput").ap(),
    }
    with tile.TileContext(nc) as tc:
        _emit(tc, io)
    nc.compile()
    _CACHE["nc"] = nc
    return nc


def make_in_maps(x, w_qkv, b_qkv, w_fc):
    """Host-side sharding: returns per-core input dicts (core = b*G + g)."""
    x = np.asarray(x, dtype=np.float32)
    w_qkv = np.asarray(w_qkv, dtype=np.float32)
    b_qkv = np.asarray(b_qkv, dtype=np.float32)
    w_fc = np.asarray(w_fc, dtype=np.float32)
    npdt = np.float32
    in_maps = []
    for b in range(B):
        xTb = np.ascontiguousarray(x[b].T).astype(npdt)
        for g in range(G):
            gs = slice(g * DG, (g + 1) * DG)
            in_maps.append({
                "xT": xTb,
                "wq": np.ascontiguousarray(
                    w_qkv[:, 0 * E:1 * E][:, gs] * (1.0 / np.sqrt(D))
                ).astype(npdt),
                "wk": np.ascontiguousarray(w_qkv[:, 1 * E:2 * E][:, gs]).astype(npdt),
                "wv": np.ascontiguousarray(w_qkv[:, 2 * E:3 * E][:, gs]).astype(npdt),
                "bq": np.ascontiguousarray(
                    b_qkv[0 * E:1 * E][gs][None, :] * (1.0 / np.sqrt(D))
                ).astype(npdt),
                "wfc": np.ascontiguousarray(w_fc[gs, :]).astype(npdt),
            })
    return in_maps


def gather(results, b_qkv, w_fc, b_fc):
    """Host-side unshard: sum group partials, transpose, add const bias."""
    b_qkv = np.asarray(b_qkv, dtype=np.float32)
    w_fc = np.asarray(w_fc, dtype=np.float32)
    b_fc = np.asarray(b_fc, dtype=np.float32)
    cbias = (b_qkv[2 * E:3 * E].astype(np.float64) @ w_fc.astype(np.float64)
             + b_fc.astype(np.float64)).astype(np.float32)
    y = np.empty((B, S, E), np.float32)
    for b in range(B):
        yT = results[b * G]["yT"] + results[b * G + 1]["yT"]
        y[b] = yT.T + cbias[None, :]
    return y


def kernel(x, w_qkv, b_qkv, w_fc, b_fc, _trace=False, _tmpdir=None):
    nc = _build()
    in_maps = make_in_maps(x, w_qkv, b_qkv, w_fc)
    res = run_bass_kernel_spmd(nc, in_maps, list(range(NCORES)),
                               trace=_trace, tmpdir=_tmpdir)
    y = gather(res.results, b_qkv, w_fc, b_fc)
    kernel.last_exec_time_ns = res.exec_time_ns
    kernel.last_res = res
    return y
